# revision 2
# baseline (speedup 1.0000x reference)
"""Trainium2 Bass kernel for a dense transformer block (RMSNorm -> causal MHA
-> residual -> RMSNorm -> SwiGLU MLP -> residual), distributed over 8
NeuronCores.

Sharding: core c handles batch b = c//2 and query parity half = c%2 (the
interleaved token slice half::2, QT=1024 query tokens per core).  K/V
projections are split between the two cores of a batch pair (each computes
K/V for 1024 contiguous tokens) and exchanged with two 2-rank AllGathers
that overlap the V projection / hq norm.

All matmul operands fp16 (full PE speed at any tile size).  The softmax
denominator is a single ones[128x128] matmul (cross-partition sum +
broadcast in one PE op) instead of a GpSimd partition_all_reduce -- the
GpSimd version cost 3.5us + library-load thrash per (head, slice) and
serialized the attention inner loop (PE sat idle 13us/head and HAM
re-throttled the clock to 1.2GHz).  Score tiles are staged one pair ahead;
PSUM rings: qps/scp/dbc share a 3-deep [P,1024] ring + 2 oacc banks.
"""

import numpy as np

import concourse.bass as bass
import concourse.bass_isa as bass_isa
import concourse.bacc as bacc
import concourse.mybir as mybir
from concourse.tile import TileContext
from concourse.bass_utils import run_bass_kernel_spmd

F32 = mybir.dt.float32
F16 = mybir.dt.float16
AF = mybir.ActivationFunctionType
ALU = mybir.AluOpType

P = 128
N_CORES = 8
EPS = 1e-6
PAIRS = [[0, 1], [2, 3], [4, 5], [6, 7]]


class CFG:
    def __init__(self, D, T, FF, QT):
        self.D, self.T, self.TD, self.FF, self.QT = D, T, D, FF, QT
        self.NS = 512
        self.DT = D // P            # contraction tiles over model dim
        self.H = self.TD // P       # heads (dh == P)
        self.KT = T // P            # key tiles
        self.NB = T // self.NS      # 512-token blocks over full sequence
        self.NBH = self.NB // 2     # blocks this core projects K/V for
        self.KTH = self.KT // 2     # key tiles per half
        self.NQS = QT // self.NS    # query slices
        self.NVS = self.TD // self.NS  # v column slabs
        self.NFT = FF // P          # ff tiles
        self.NDCT = D // P          # output col tiles
        self.stride = T // QT       # query interleave stride
        self.NSLOT = self.stride * self.NS // P  # partial (diagonal) k tiles / slice
        self.ISQ = 1.0 / float(np.sqrt(P))

    def nkt(self, qs):
        return min((qs + 1) * self.stride * self.NS // P, self.KT)

    def kt0(self, qs):
        # first partially-masked k tile for query slice qs
        return self.stride * qs * self.NS // P


FULL = CFG(D=2048, T=2048, FF=8192, QT=1024)


def build(cfg):
    D, T, TD, FF, QT, NS = cfg.D, cfg.T, cfg.TD, cfg.FF, cfg.QT, cfg.NS
    DT, H, KT, NB, NQS = cfg.DT, cfg.H, cfg.KT, cfg.NB, cfg.NQS
    NVS, NFT, NDCT, NSLOT = cfg.NVS, cfg.NFT, cfg.NDCT, cfg.NSLOT
    NBH, KTH = cfg.NBH, cfg.KTH
    HPS = NS // P               # heads per v slab
    stride = cfg.stride

    nc = bacc.Bacc("TRN2", target_bir_lowering=False, num_devices=N_CORES)

    # ---- inputs (pre-tiled on host, fp16) ----
    # x_in: this core's half of the batch tokens (for K/V projection)
    x_in = nc.dram_tensor("x_in", [DT, P, NBH * NS], F16, kind="ExternalInput")
    xq_in = nc.dram_tensor("xq_in", [DT, P, QT], F16, kind="ExternalInput")
    m01_in = nc.dram_tensor("m01_in", [P, NSLOT, NS], F16, kind="ExternalInput")
    wq_in = nc.dram_tensor("wq_in", [H, P, DT, P], F16, kind="ExternalInput")
    wk_in = nc.dram_tensor("wk_in", [H, P, DT, P], F16, kind="ExternalInput")
    wv_in = nc.dram_tensor("wv_in", [NVS, P, DT, NS], F16, kind="ExternalInput")
    wo_in = nc.dram_tensor("wo_in", [NDCT, P, H, P], F16, kind="ExternalInput")
    wg_in = nc.dram_tensor("wg_in", [NFT, P, DT, P], F16, kind="ExternalInput")
    wu_in = nc.dram_tensor("wu_in", [NFT, P, DT, P], F16, kind="ExternalInput")
    wd_in = nc.dram_tensor("wd_in", [NDCT, P, NFT, P], F16, kind="ExternalInput")
    y_out = nc.dram_tensor("y_out", [NDCT, P, QT], F32, kind="ExternalOutput")

    # ---- scratch DRAM: local K/V halves + pair-gathered full K/V ----
    k_half = nc.dram_tensor("k_half", [H, P, NBH * NS], F16)
    NVH = NVS // 2
    v_halfA = nc.dram_tensor("v_halfA", [NVH, KTH, P, NS], F16)
    v_halfB = nc.dram_tensor("v_halfB", [NVH, KTH, P, NS], F16)
    k_gath = nc.dram_tensor("k_gath", [2, H, P, NBH * NS], F16)
    v_gathA = nc.dram_tensor("v_gathA", [2, NVH, KTH, P, NS], F16)
    v_gathB = nc.dram_tensor("v_gathB", [2, NVH, KTH, P, NS], F16)

    with TileContext(nc) as tc, \
            nc.allow_low_precision("fp16 softmax/norm sums; tol 2e-2"):
        pc = tc.alloc_tile_pool(name="const", bufs=1)
        ones_c = pc.tile([P, 1], F16, tag="ones_c")
        nc.vector.memset(ones_c[:], 1.0)
        ones_r = pc.tile([1, P], F16, tag="ones_r")
        nc.vector.memset(ones_r[:], 1.0)
        ones_sq = pc.tile([P, P], F16, tag="ones_sq")
        nc.vector.memset(ones_sq[:], 1.0)
        epsT = pc.tile([1, 1], F32, tag="eps")
        nc.vector.memset(epsT[:], EPS)
        m01 = pc.tile([P, NSLOT, NS], F16, tag="m01")

        # persistent activations
        pax = tc.alloc_tile_pool(name="ax", bufs=1)      # hq + xq (A -> P3)
        xq_sb = pax.tile([P, DT, QT], F16, tag="xq")
        hq = pax.tile([P, DT, QT], F16, tag="hq")

        # ========== A+B: rmsnorm + h for this half, K/V proj, exchange ====
        ph = tc.alloc_tile_pool(name="h", bufs=1)        # h blocks (A -> B)
        h_blk = [ph.tile([P, DT, NS], F16, tag=f"h{tb}", name=f"h{tb}")
                 for tb in range(NBH)]
        with nc.named_scope("AB"):
            with tc.tile_pool(name="pa", bufs=2) as pa, \
                 tc.tile_pool(name="pb", bufs=3) as pb, \
                 tc.tile_pool(name="pbk", bufs=2) as pbk, \
                 tc.tile_pool(name="pa_ps", bufs=2, space="PSUM") as pa_ps, \
                 tc.tile_pool(name="pa_bc", bufs=2, space="PSUM") as pa_bc, \
                 tc.tile_pool(name="pb_k", bufs=2, space="PSUM") as pb_k:

                def norm_into(dst, src, t0):
                    """rmsnorm scale of 512 tokens of src -> dst (both
                    [P, DT, *] views at column t0)."""
                    ssp = pa_ps.tile([1, NS], F32, tag="ssp")
                    for c in range(DT // 4):
                        sq = pa.tile([P, 4, NS], F16, tag="sq")
                        nc.scalar.activation(
                            sq[:], src[:, 4 * c:4 * c + 4, t0:t0 + NS],
                            AF.Square)
                        for j in range(4):
                            dt = 4 * c + j
                            nc.tensor.matmul(ssp[:], ones_c[:], sq[:, j, :],
                                             start=(dt == 0),
                                             stop=(dt == DT - 1))
                    srow = pa.tile([1, NS], F32, tag="srow")
                    nc.scalar.activation(srow[:], ssp[:], AF.Sqrt,
                                         scale=1.0 / D, bias=epsT[:])
                    rec32 = pa.tile([1, NS], F32, tag="rec32")
                    nc.vector.reciprocal_approx_fast(rec32[:], srow[:])
                    rec16 = pa.tile([1, NS], F16, tag="rec16")
                    nc.scalar.copy(rec16[:], rec32[:])
                    bcp = pa_bc.tile([P, NS], F32, tag="bcp")
                    nc.tensor.matmul(bcp[:], ones_r[:], rec16[:],
                                     start=True, stop=True)
                    bcs = pa.tile([P, NS], F16, tag="bcs")
                    nc.scalar.copy(bcs[:], bcp[:])
                    for dt in range(DT):
                        nc.vector.tensor_tensor(dst[:, dt, t0:t0 + NS],
                                                src[:, dt, t0:t0 + NS],
                                                bcs[:], ALU.mult)

                # load + norm this core's two 512-token blocks
                xbs = []
                for tb in range(NBH):
                    xb = pa.tile([P, DT, NS], F16, tag="xb", name=f"xb{tb}")
                    nc.sync.dma_start(
                        out=xb[:],
                        in_=x_in[:, :, tb * NS:(tb + 1) * NS].rearrange(
                            "a p c -> p a c"))
                    xbs.append(xb)
                nc.sync.dma_start(
                    out=xq_sb[:], in_=xq_in.rearrange("a p c -> p a c"))
                nc.sync.dma_start(out=m01[:], in_=m01_in[:])
                for tb in range(NBH):
                    norm_into(h_blk[tb], xbs[tb], 0)

                # K projection for this half: one weight tile covers both
                # 512-token blocks
                for hh in range(H):
                    wk = pbk.tile([P, DT, P], F16, tag="wk")
                    nc.sync.dma_start(out=wk[:], in_=wk_in[hh])
                    kps = pb_k.tile([P, 2, NS], F32, tag="kps")
                    for dt in range(DT):
                        st, sp = (dt == 0), (dt == DT - 1)
                        for b in range(NBH):
                            nc.tensor.matmul(
                                kps[:, b, :], wk[:, dt, :],
                                h_blk[b][:, dt, :],
                                start=st, stop=sp)
                    kcp = pb.tile([P, 2 * NS], F16, tag="kcp")
                    nc.scalar.copy(kcp[:], kps.rearrange("p a b -> p (a b)"))
                    nc.sync.dma_start(out=k_half[hh], in_=kcp[:])

                # exchange K halves while V projection runs
                nc.gpsimd.collective_compute(
                    "AllGather", mybir.AluOpType.bypass,
                    replica_groups=PAIRS,
                    ins=[k_half[:, :, :].opt()],
                    outs=[k_gath[:, :, :, :].opt()])

            # ---- V projection for this half; hq norms interleaved so their
            # ACT/DVE chains hide under the V matmuls ----
            with tc.tile_pool(name="pbv2", bufs=3) as pb2, \
                 tc.tile_pool(name="pbw", bufs=2) as pbw, \
                 tc.tile_pool(name="pa2", bufs=2) as pa2, \
                 tc.tile_pool(name="pa2_ps", bufs=2, space="PSUM") as pa2_ps, \
                 tc.tile_pool(name="pb_v", bufs=3, space="PSUM") as pb_v:
                def norm_hq(t0):
                    ssp = pa2_ps.tile([1, NS], F32, tag="ssp")
                    for c in range(DT // 4):
                        sq = pa2.tile([P, 4, NS], F16, tag="sq")
                        nc.scalar.activation(
                            sq[:], xq_sb[:, 4 * c:4 * c + 4, t0:t0 + NS],
                            AF.Square)
                        for j in range(4):
                            dt = 4 * c + j
                            nc.tensor.matmul(ssp[:], ones_c[:], sq[:, j, :],
                                             start=(dt == 0),
                                             stop=(dt == DT - 1))
                    srow = pa2.tile([1, NS], F32, tag="srow")
                    nc.scalar.activation(srow[:], ssp[:], AF.Sqrt,
                                         scale=1.0 / D, bias=epsT[:])
                    rec32 = pa2.tile([1, NS], F32, tag="rec32")
                    nc.vector.reciprocal_approx_fast(rec32[:], srow[:])
                    rec16 = pa2.tile([1, NS], F16, tag="rec16")
                    nc.scalar.copy(rec16[:], rec32[:])
                    bcp = pa2_ps.tile([P, NS], F32, tag="bcp")
                    nc.tensor.matmul(bcp[:], ones_r[:], rec16[:],
                                     start=True, stop=True)
                    bcs = pa2.tile([P, NS], F16, tag="bcs")
                    nc.scalar.copy(bcs[:], bcp[:])
                    for dt in range(DT):
                        nc.vector.tensor_tensor(hq[:, dt, t0:t0 + NS],
                                                xq_sb[:, dt, t0:t0 + NS],
                                                bcs[:], ALU.mult)

                for vs in range(NVS):
                    v_half = v_halfA if vs < NVH else v_halfB
                    wv = pbw.tile([P, DT, NS], F16, tag="wv")
                    nc.sync.dma_start(out=wv[:], in_=wv_in[vs])
                    for kt in range(KTH):
                        tb, off = divmod(kt * P, NS)
                        vps = pb_v.tile([P, NS], F32, tag="vps")
                        for dt in range(DT):
                            nc.tensor.matmul(
                                vps[:], h_blk[tb][:, dt, off:off + P],
                                wv[:, dt, :],
                                start=(dt == 0), stop=(dt == DT - 1))
                        vcp = pb2.tile([P, NS], F16, tag="vcp")
                        nc.scalar.copy(vcp[:], vps[:])
                        nc.sync.dma_start(out=v_half[vs % NVH, kt],
                                          in_=vcp[:])
                    if vs < NQS:
                        norm_hq(vs * NS)
                    if vs == NVH - 1:
                        # first half of V (heads 0..7) exchanges while the
                        # second half projects
                        nc.gpsimd.collective_compute(
                            "AllGather", mybir.AluOpType.bypass,
                            replica_groups=PAIRS,
                            ins=[v_halfA[:, :, :, :].opt()],
                            outs=[v_gathA[:, :, :, :, :].opt()])
                nc.gpsimd.collective_compute(
                    "AllGather", mybir.AluOpType.bypass,
                    replica_groups=PAIRS,
                    ins=[v_halfB[:, :, :, :].opt()],
                    outs=[v_gathB[:, :, :, :, :].opt()])
        ph.release()

        # ================= P2: Q projection + causal attention ===========
        po = tc.alloc_tile_pool(name="o", bufs=1)        # o (P2 -> P3)
        o_w = [po.tile([P, H, NS], F16, tag=f"o{ws}", name=f"o{ws}")
               for ws in range(NQS)]
        with nc.named_scope("P2"):
            with tc.tile_pool(name="p2", bufs=3) as p2, \
                 tc.tile_pool(name="p2kv", bufs=2) as p2kv, \
                 tc.tile_pool(name="p2pex", bufs=4) as p2pex, \
                 tc.tile_pool(name="p2w", bufs=2) as p2w, \
                 tc.tile_pool(name="p2mm", bufs=3, space="PSUM") as p2mm, \
                 tc.tile_pool(name="p2acc", bufs=2, space="PSUM") as p2acc:
                def qproj(hh):
                    """Q projection for head hh, interleaved with the
                    previous head's softmax chains to keep the PE fed."""
                    wq = p2w.tile([P, DT, P], F16, tag="wq")
                    nc.sync.dma_start(out=wq[:], in_=wq_in[hh])
                    qps = p2mm.tile([P, NQS * NS], F32, tag="mm")
                    for ws in range(NQS):
                        for dt in range(DT):
                            nc.tensor.matmul(
                                qps[:, ws * NS:(ws + 1) * NS],
                                wq[:, dt, :], hq[:, dt, ws * NS:(ws + 1) * NS],
                                start=(dt == 0), stop=(dt == DT - 1))
                    qh = p2.tile([P, QT], F16, tag="qh")
                    nc.scalar.copy(qh[:], qps[:])
                    return qh

                qh_cur = qproj(0)
                for hh in range(H):
                    kh = p2kv.tile([P, T], F16, tag="kh")
                    for g in range(2):
                        nc.sync.dma_start(
                            out=kh[:, g * KTH * P:(g + 1) * KTH * P],
                            in_=k_gath[g, hh])
                    vh = p2kv.tile([P, KT, P], F16, tag="vh")
                    voff = (hh % HPS) * P
                    v_gath = v_gathA if hh < H // 2 else v_gathB
                    vslab = (hh // HPS) % NVH
                    for g in range(2):
                        nc.sync.dma_start(
                            out=vh[:, g * KTH:(g + 1) * KTH, :],
                            in_=v_gath[g, vslab].rearrange(
                                "t p c -> p t c")[:, :, voff:voff + P])
                    qh = qh_cur
                    qh_cur = qproj(hh + 1) if hh + 1 < H else None
                    for qs in range(NQS):
                        nkt = cfg.nkt(qs)
                        kt0 = cfg.kt0(qs)
                        npair = nkt // 2
                        oacc = p2acc.tile([P, NS], F32, tag="oacc")
                        dsum = p2.tile([P, NS], F16, tag="dsum")
                        scps = {}

                        def emit_scp(kp):
                            scp = p2mm.tile([P, 2, NS], F32, tag="mm")
                            for half in range(2):
                                kt = 2 * kp + half
                                nc.tensor.matmul(
                                    scp[:, half, :], kh[:, kt * P:(kt + 1) * P],
                                    qh[:, qs * NS:(qs + 1) * NS],
                                    start=True, stop=True)
                            scps[kp] = scp

                        def emit_rest(kp):
                            scp = scps.pop(kp)
                            pex = p2pex.tile([P, 2, NS], F16, tag="pex")
                            nc.scalar.activation(pex[:], scp[:], AF.Exp,
                                                 scale=cfg.ISQ)
                            masked = 2 * kp >= kt0
                            if masked:
                                s = 2 * kp - kt0
                                pexm = p2pex.tile([P, 2, NS], F16, tag="pexm")
                                nc.vector.tensor_tensor(
                                    pexm[:], pex[:], m01[:, s:s + 2, :],
                                    ALU.mult)
                                pex = pexm
                            if kp == 0:
                                nc.vector.tensor_tensor(
                                    dsum[:], pex[:, 0, :], pex[:, 1, :],
                                    ALU.add)
                            else:
                                for half in range(2):
                                    nc.vector.tensor_tensor(
                                        dsum[:], dsum[:], pex[:, half, :],
                                        ALU.add)
                            for half in range(2):
                                kt = 2 * kp + half
                                nc.tensor.matmul(
                                    oacc[:], vh[:, kt, :], pex[:, half, :],
                                    start=(kt == 0), stop=(kt == nkt - 1))

                        emit_scp(0)
                        if npair > 1:
                            emit_scp(1)
                        for kp in range(npair):
                            if kp + 2 < npair:
                                emit_scp(kp + 2)
                            emit_rest(kp)
                        # softmax denominator: ones[128x128] matmul sums the
                        # partition dim AND broadcasts in one PE op.  Lives
                        # in the oacc ring so the scp ring keeps 2-ahead
                        # staging depth.
                        dbc_t = p2acc.tile([P, NS], F32, tag="oacc")
                        nc.tensor.matmul(dbc_t[:], ones_sq[:], dsum[:],
                                         start=True, stop=True)
                        recb = p2.tile([P, NS], F32, tag="recb")
                        nc.vector.reciprocal_approx_fast(recb[:], dbc_t[:])
                        nc.vector.tensor_tensor(o_w[qs][:, hh, :], oacc[:],
                                                recb[:], ALU.mult)

        # ================= P3: out-proj + residual + norm2 ===============
        # right-side stack: lifetime (P3 -> P5) crosses po's release
        px2 = tc.alloc_tile_pool(name="x2h2", bufs=1, side="right")
        x2 = px2.tile([P, NDCT, QT], F16, tag="x2")
        h2 = px2.tile([P, DT, QT], F16, tag="h2")
        with nc.named_scope("P3"):
            with tc.tile_pool(name="p3", bufs=2) as p3, \
                 tc.tile_pool(name="p3w", bufs=2) as p3w, \
                 tc.tile_pool(name="p3mm", bufs=2, space="PSUM") as p3mm, \
                 tc.tile_pool(name="p3s", bufs=1, space="PSUM") as p3s, \
                 tc.tile_pool(name="p3bc", bufs=2, space="PSUM") as p3bc:
                ssp2 = p3s.tile([1, QT], F32, tag="ssp2")

                def stat2(dct, sq2):
                    for ws in range(NQS):
                        nc.tensor.matmul(ssp2[:, ws * NS:(ws + 1) * NS],
                                         ones_c[:],
                                         sq2[:, ws * NS:(ws + 1) * NS],
                                         start=(dct == 0),
                                         stop=(dct == NDCT - 1))

                pend = []
                for dct in range(NDCT):
                    wo = p3w.tile([P, H, P], F16, tag="wo")
                    nc.sync.dma_start(out=wo[:], in_=wo_in[dct])
                    ops = p3mm.tile([P, NQS, NS], F32, tag="ops")
                    for ws in range(NQS):
                        for hh in range(H):
                            nc.tensor.matmul(
                                ops[:, ws, :], wo[:, hh, :], o_w[ws][:, hh, :],
                                start=(hh == 0), stop=(hh == H - 1))
                    nc.vector.tensor_tensor(
                        x2[:, dct, :],
                        ops.rearrange("p a b -> p (a b)"),
                        xq_sb[:, dct, :], ALU.add)
                    sq2 = p3.tile([P, QT], F16, tag="sq2", bufs=4)
                    nc.scalar.activation(sq2[:], x2[:, dct, :], AF.Square)
                    # delay the tiny stat matmuls two dcts so the PE isn't
                    # held hostage to the DVE->Act chain of the current dct
                    pend.append((dct, sq2))
                    if len(pend) >= 3:
                        stat2(*pend.pop(0))
                for it in pend:
                    stat2(*it)
                for ws in range(NQS):
                    q0 = ws * NS
                    srow2 = p3.tile([1, NS], F32, tag="srow2")
                    nc.scalar.activation(srow2[:], ssp2[:, q0:q0 + NS],
                                         AF.Sqrt, scale=1.0 / D, bias=epsT[:])
                    rec232 = p3.tile([1, NS], F32, tag="rec232")
                    nc.vector.reciprocal_approx_fast(rec232[:], srow2[:])
                    rec2 = p3.tile([1, NS], F16, tag="rec2")
                    nc.scalar.copy(rec2[:], rec232[:])
                    bc2 = p3bc.tile([P, NS], F32, tag="bc2")
                    nc.tensor.matmul(bc2[:], ones_r[:], rec2[:],
                                     start=True, stop=True)
                    bc2s = p3.tile([P, NS], F16, tag="bc2s")
                    nc.scalar.copy(bc2s[:], bc2[:])
                    for dt in range(DT):
                        nc.vector.tensor_tensor(h2[:, dt, q0:q0 + NS],
                                                x2[:, dt, q0:q0 + NS],
                                                bc2s[:], ALU.mult)
        po.release()
        pax.release()

        # ================= P5: SwiGLU MLP + residual =====================
        with nc.named_scope("P5"):
            with tc.tile_pool(name="p5", bufs=2) as p5, \
                 tc.tile_pool(name="p5w", bufs=2) as p5w, \
                 tc.tile_pool(name="p5mt", bufs=1) as p5mt, \
                 tc.tile_pool(name="p5gu", bufs=2, space="PSUM") as p5gu, \
                 tc.tile_pool(name="p5d", bufs=3, space="PSUM") as p5d:
                for ws in range(NQS):
                    q0 = ws * NS
                    mt = p5mt.tile([P, NFT, NS], F16, tag="mt")
                    for ft in range(NFT):
                        wg = p5w.tile([P, DT, P], F16, tag="wg")
                        nc.sync.dma_start(out=wg[:], in_=wg_in[ft])
                        wu = p5w.tile([P, DT, P], F16, tag="wu")
                        nc.sync.dma_start(out=wu[:], in_=wu_in[ft])
                        guw = p5gu.tile([P, 2, NS], F32, tag="guw")
                        for dt in range(DT):
                            nc.tensor.matmul(
                                guw[:, 0, :], wg[:, dt, :],
                                h2[:, dt, q0:q0 + NS],
                                start=(dt == 0), stop=(dt == DT - 1))
                        for dt in range(DT):
                            nc.tensor.matmul(
                                guw[:, 1, :], wu[:, dt, :],
                                h2[:, dt, q0:q0 + NS],
                                start=(dt == 0), stop=(dt == DT - 1))
                        sg = p5.tile([P, NS], F16, tag="sg")
                        nc.scalar.activation(sg[:], guw[:, 0, :], AF.Silu)
                        nc.vector.tensor_tensor(mt[:, ft, :], sg[:],
                                                guw[:, 1, :], ALU.mult)
                    for dct in range(NDCT):
                        wd = p5w.tile([P, NFT, P], F16, tag="wd")
                        nc.sync.dma_start(out=wd[:], in_=wd_in[dct])
                        dps = p5d.tile([P, NS], F32, tag="dacc")
                        for ft in range(NFT):
                            nc.tensor.matmul(dps[:], wd[:, ft, :],
                                             mt[:, ft, :],
                                             start=(ft == 0),
                                             stop=(ft == NFT - 1))
                        yt = p5.tile([P, NS], F32, tag="yt")
                        nc.vector.tensor_tensor(yt[:], dps[:],
                                                x2[:, dct, q0:q0 + NS],
                                                ALU.add)
                        nc.sync.dma_start(out=y_out[dct][:, q0:q0 + NS],
                                          in_=yt[:])
        px2.release()
        pc.release()

    nc.compile()
    return nc


# --------------------------------------------------------------------------
# Host side
# --------------------------------------------------------------------------

_NC_CACHE = {}


def _get_nc(cfg):
    key = (cfg.D, cfg.T, cfg.FF, cfg.QT)
    if key not in _NC_CACHE:
        _NC_CACHE[key] = build(cfg)
    return _NC_CACHE[key]


def _tile_lhs(a, ncols):
    # [Din, Cout] -> [Cout/ncols, P, Din/P, ncols]
    d, c = a.shape
    return np.ascontiguousarray(
        a.reshape(d // P, P, c // ncols, ncols).transpose(2, 1, 0, 3))


def prep_weights(cfg, w_qkv, w_out, w_gate, w_up, w_down, ln1, ln2):
    D, TD, FF, NS = cfg.D, cfg.TD, cfg.FF, cfg.NS
    f32, f16 = np.float32, np.float16
    w_qkv_f = (np.asarray(w_qkv, f32) * np.asarray(ln1, f32)[None, :])
    wqT = w_qkv_f[0:TD].T
    wkT = w_qkv_f[TD:2 * TD].T
    wvT = w_qkv_f[2 * TD:3 * TD].T
    woT = np.asarray(w_out, f32).T            # [TD, D]
    wgT = (np.asarray(w_gate, f32) * np.asarray(ln2, f32)[None, :]).T
    wuT = (np.asarray(w_up, f32) * np.asarray(ln2, f32)[None, :]).T
    wdT = np.asarray(w_down, f32).T           # [FF, D]

    wd_in = np.ascontiguousarray(
        wdT.reshape(cfg.NFT, P, cfg.NDCT, P).transpose(2, 1, 0, 3))
    return dict(
        wq_in=_tile_lhs(wqT, P).astype(f16),
        wk_in=_tile_lhs(wkT, P).astype(f16),
        wv_in=_tile_lhs(wvT, NS).astype(f16),
        wo_in=_tile_lhs(woT, P).astype(f16),
        wg_in=_tile_lhs(wgT, P).astype(f16),
        wu_in=_tile_lhs(wuT, P).astype(f16),
        wd_in=wd_in.astype(f16),
    )


def prep_core_inputs(cfg, xb, parity, wdict):
    """Per-core tensors for batch slice xb [T, D]; query tokens are the
    interleaved slice parity::stride; K/V tokens are the contiguous half
    [parity*T/2, (parity+1)*T/2)."""
    T, D, QT, NS = cfg.T, cfg.D, cfg.QT, cfg.NS
    stride = cfg.stride
    f16 = np.float16
    half = T // 2
    xT = np.ascontiguousarray(np.asarray(xb, np.float32).T)   # [D, T]
    x_in = np.ascontiguousarray(
        xT[:, parity * half:(parity + 1) * half]).reshape(
        cfg.DT, P, half).astype(f16)
    xq_in = np.ascontiguousarray(
        xT[:, parity::stride]).reshape(cfg.DT, P, QT).astype(f16)
    # 0/1 mask for the NSLOT diagonal k tiles of every query slice:
    # slot s, row ki, col q allowed iff 128*s + ki <= stride*q + parity
    ki = np.arange(P)[:, None, None]
    s = np.arange(cfg.NSLOT)[None, :, None]
    q = np.arange(NS)[None, None, :]
    m01_in = ((P * s + ki) <= (stride * q + parity)).astype(f16)
    out = dict(x_in=x_in, xq_in=xq_in, m01_in=m01_in)
    out.update(wdict)
    return out


def run(cfg, x, w_qkv, w_out, w_gate, w_up, w_down, ln1, ln2):
    nc = _get_nc(cfg)
    wdict = prep_weights(cfg, w_qkv, w_out, w_gate, w_up, w_down, ln1, ln2)
    x = np.asarray(x, np.float32)
    Bc = x.shape[0]
    in_maps = []
    for c in range(N_CORES):
        b, half = divmod(c, 2)
        in_maps.append(prep_core_inputs(cfg, x[b % Bc], half, wdict))
    res = run_bass_kernel_spmd(nc, in_maps, list(range(N_CORES)))
    y = np.empty((Bc, cfg.T, cfg.D), np.float32)
    for c in range(N_CORES):
        b, parity = divmod(c, 2)
        if b < Bc:
            yc = res.results[c]["y_out"].reshape(cfg.D, cfg.QT)
            y[b, parity::cfg.stride, :] = yc.T
    return y


def kernel(x, w_qkv, w_out, w_gate, w_up, w_down, ln1, ln2):
    return run(FULL, x, w_qkv, w_out, w_gate, w_up, w_down, ln1, ln2)


# revision 4
# speedup vs baseline: 1.1771x; 1.1771x over previous
"""Trainium2 Bass kernel for a dense transformer block (RMSNorm -> causal MHA
-> residual -> RMSNorm -> SwiGLU MLP -> residual), distributed over 8
NeuronCores.

Sharding: core c handles batch b = c//2 and query parity half = c%2 (the
interleaved token slice half::2, QT=1024 query tokens per core).  K/V
projections are split between the two cores of a batch pair (each computes
K/V for 1024 contiguous tokens) and exchanged with two 2-rank AllGathers
that overlap the V projection / hq norm.

All matmul operands fp16 (full PE speed at any tile size; fp8 was measured
to break the 2e-2 error budget on any single large matmul).  The softmax
denominator is a single ones[128x128] matmul (cross-partition sum +
broadcast in one PE op) instead of a GpSimd partition_all_reduce -- the
GpSimd version cost 3.5us + library-load thrash per (head, slice) and
serialized the attention inner loop (PE sat idle 13us/head and HAM
re-throttled the clock to 1.2GHz).  Score tiles are staged three key-tiles
ahead and the next head's Q-projection matmuls are pumped one-by-one into
the attention stream as PE fillers, so the PE never waits on the exp chain.
PSUM budget (8 banks): 4 score staging + 2 qproj + 2 oacc/denominator.
The hq norm chains hide under the K-projection matmuls.
"""

import numpy as np

import concourse.bass as bass
import concourse.bass_isa as bass_isa
import concourse.bacc as bacc
import concourse.mybir as mybir
from concourse.tile import TileContext
from concourse.bass_utils import run_bass_kernel_spmd

F32 = mybir.dt.float32
F16 = mybir.dt.float16
AF = mybir.ActivationFunctionType
ALU = mybir.AluOpType

P = 128
N_CORES = 8
EPS = 1e-6
PAIRS = [[0, 1], [2, 3], [4, 5], [6, 7]]


class CFG:
    def __init__(self, D, T, FF, QT):
        self.D, self.T, self.TD, self.FF, self.QT = D, T, D, FF, QT
        self.NS = 512
        self.DT = D // P            # contraction tiles over model dim
        self.H = self.TD // P       # heads (dh == P)
        self.KT = T // P            # key tiles
        self.NB = T // self.NS      # 512-token blocks over full sequence
        self.NBH = self.NB // 2     # blocks this core projects K/V for
        self.KTH = self.KT // 2     # key tiles per half
        self.NQS = QT // self.NS    # query slices
        self.NVS = self.TD // self.NS  # v column slabs
        self.NFT = FF // P          # ff tiles
        self.NDCT = D // P          # output col tiles
        self.stride = T // QT       # query interleave stride
        self.NSLOT = self.stride * self.NS // P  # partial (diagonal) k tiles / slice
        self.ISQ = 1.0 / float(np.sqrt(P))

    def nkt(self, qs):
        return min((qs + 1) * self.stride * self.NS // P, self.KT)

    def kt0(self, qs):
        # first partially-masked k tile for query slice qs
        return self.stride * qs * self.NS // P


FULL = CFG(D=2048, T=2048, FF=8192, QT=1024)


def build(cfg):
    D, T, TD, FF, QT, NS = cfg.D, cfg.T, cfg.TD, cfg.FF, cfg.QT, cfg.NS
    DT, H, KT, NB, NQS = cfg.DT, cfg.H, cfg.KT, cfg.NB, cfg.NQS
    NVS, NFT, NDCT, NSLOT = cfg.NVS, cfg.NFT, cfg.NDCT, cfg.NSLOT
    NBH, KTH = cfg.NBH, cfg.KTH
    HPS = NS // P               # heads per v slab
    stride = cfg.stride

    nc = bacc.Bacc("TRN2", target_bir_lowering=False, num_devices=N_CORES)

    # ---- inputs (pre-tiled on host, fp16) ----
    # x_in: this core's half of the batch tokens (for K/V projection)
    x_in = nc.dram_tensor("x_in", [DT, P, NBH * NS], F16, kind="ExternalInput")
    xq_in = nc.dram_tensor("xq_in", [DT, P, QT], F16, kind="ExternalInput")
    m01_in = nc.dram_tensor("m01_in", [P, NSLOT, NS], F16, kind="ExternalInput")
    wq_in = nc.dram_tensor("wq_in", [H, P, DT, P], F16, kind="ExternalInput")
    wk_in = nc.dram_tensor("wk_in", [H, P, DT, P], F16, kind="ExternalInput")
    wv_in = nc.dram_tensor("wv_in", [NVS, P, DT, NS], F16, kind="ExternalInput")
    wo_in = nc.dram_tensor("wo_in", [NDCT, P, H, P], F16, kind="ExternalInput")
    wg_in = nc.dram_tensor("wg_in", [NFT, P, DT, P], F16, kind="ExternalInput")
    wu_in = nc.dram_tensor("wu_in", [NFT, P, DT, P], F16, kind="ExternalInput")
    wd_in = nc.dram_tensor("wd_in", [NDCT, P, NFT, P], F16, kind="ExternalInput")
    y_out = nc.dram_tensor("y_out", [NDCT, P, QT], F32, kind="ExternalOutput")

    # ---- scratch DRAM: local K/V halves + pair-gathered full K/V ----
    k_half = nc.dram_tensor("k_half", [H, P, NBH * NS], F16)
    NVH = NVS // 2
    v_halfA = nc.dram_tensor("v_halfA", [NVH, KTH, P, NS], F16)
    v_halfB = nc.dram_tensor("v_halfB", [NVH, KTH, P, NS], F16)
    k_gath = nc.dram_tensor("k_gath", [2, H, P, NBH * NS], F16)
    v_gathA = nc.dram_tensor("v_gathA", [2, NVH, KTH, P, NS], F16)
    v_gathB = nc.dram_tensor("v_gathB", [2, NVH, KTH, P, NS], F16)

    with TileContext(nc) as tc, \
            nc.allow_low_precision("fp16 softmax/norm sums; tol 2e-2"):
        pc = tc.alloc_tile_pool(name="const", bufs=1)
        ones_c = pc.tile([P, 1], F16, tag="ones_c")
        nc.vector.memset(ones_c[:], 1.0)
        ones_r = pc.tile([1, P], F16, tag="ones_r")
        nc.vector.memset(ones_r[:], 1.0)
        ones_sq = pc.tile([P, P], F16, tag="ones_sq")
        nc.vector.memset(ones_sq[:], 1.0)
        epsT = pc.tile([1, 1], F32, tag="eps")
        nc.vector.memset(epsT[:], EPS)
        m01 = pc.tile([P, NSLOT, NS], F16, tag="m01")

        # persistent activations
        pax = tc.alloc_tile_pool(name="ax", bufs=1)      # hq + xq (A -> P3)
        xq_sb = pax.tile([P, DT, QT], F16, tag="xq")
        hq = pax.tile([P, DT, QT], F16, tag="hq")

        # ========== A+B: rmsnorm + h for this half, K/V proj, exchange ====
        ph = tc.alloc_tile_pool(name="h", bufs=1)        # h blocks (A -> B)
        h_blk = [ph.tile([P, DT, NS], F16, tag=f"h{tb}", name=f"h{tb}")
                 for tb in range(NBH)]
        with nc.named_scope("AB"):
            with tc.tile_pool(name="pa", bufs=2) as pa, \
                 tc.tile_pool(name="pb", bufs=3) as pb, \
                 tc.tile_pool(name="pbk", bufs=2) as pbk, \
                 tc.tile_pool(name="pa_ps", bufs=2, space="PSUM") as pa_ps, \
                 tc.tile_pool(name="pa_bc", bufs=2, space="PSUM") as pa_bc, \
                 tc.tile_pool(name="pb_k", bufs=2, space="PSUM") as pb_k:

                def norm_into(dst, src, t0):
                    """rmsnorm scale of 512 tokens of src -> dst (both
                    [P, DT, *] views at column t0)."""
                    ssp = pa_ps.tile([1, NS], F32, tag="ssp")
                    for c in range(DT // 4):
                        sq = pa.tile([P, 4, NS], F16, tag="sq")
                        nc.scalar.activation(
                            sq[:], src[:, 4 * c:4 * c + 4, t0:t0 + NS],
                            AF.Square)
                        for j in range(4):
                            dt = 4 * c + j
                            nc.tensor.matmul(ssp[:], ones_c[:], sq[:, j, :],
                                             start=(dt == 0),
                                             stop=(dt == DT - 1))
                    srow = pa.tile([1, NS], F32, tag="srow")
                    nc.scalar.activation(srow[:], ssp[:], AF.Sqrt,
                                         scale=1.0 / D, bias=epsT[:])
                    rec32 = pa.tile([1, NS], F32, tag="rec32")
                    nc.vector.reciprocal_approx_fast(rec32[:], srow[:])
                    rec16 = pa.tile([1, NS], F16, tag="rec16")
                    nc.scalar.copy(rec16[:], rec32[:])
                    bcp = pa_bc.tile([P, NS], F32, tag="bcp")
                    nc.tensor.matmul(bcp[:], ones_r[:], rec16[:],
                                     start=True, stop=True)
                    bcs = pa.tile([P, NS], F16, tag="bcs")
                    nc.scalar.copy(bcs[:], bcp[:])
                    for dt in range(DT):
                        nc.vector.tensor_tensor(dst[:, dt, t0:t0 + NS],
                                                src[:, dt, t0:t0 + NS],
                                                bcs[:], ALU.mult)

                # load + norm this core's two 512-token blocks
                xbs = []
                for tb in range(NBH):
                    xb = pa.tile([P, DT, NS], F16, tag="xb", name=f"xb{tb}")
                    nc.sync.dma_start(
                        out=xb[:],
                        in_=x_in[:, :, tb * NS:(tb + 1) * NS].rearrange(
                            "a p c -> p a c"))
                    xbs.append(xb)
                nc.sync.dma_start(
                    out=xq_sb[:], in_=xq_in.rearrange("a p c -> p a c"))
                nc.sync.dma_start(out=m01[:], in_=m01_in[:])
                for tb in range(NBH):
                    norm_into(h_blk[tb], xbs[tb], 0)

                # K projection for this half: one weight tile covers both
                # 512-token blocks.  The hq norm chains (ACT/DVE-heavy) are
                # emitted mid-loop so the K matmuls cover their latency.
                for hh in range(H):
                    wk = pbk.tile([P, DT, P], F16, tag="wk")
                    nc.sync.dma_start(out=wk[:], in_=wk_in[hh])
                    kps = pb_k.tile([P, 2, NS], F32, tag="kps")
                    for dt in range(DT):
                        st, sp = (dt == 0), (dt == DT - 1)
                        for b in range(NBH):
                            nc.tensor.matmul(
                                kps[:, b, :], wk[:, dt, :],
                                h_blk[b][:, dt, :],
                                start=st, stop=sp)
                    kcp = pb.tile([P, 2 * NS], F16, tag="kcp")
                    nc.scalar.copy(kcp[:], kps.rearrange("p a b -> p (a b)"))
                    nc.sync.dma_start(out=k_half[hh], in_=kcp[:])
                    if hh == 2:
                        norm_into(hq, xq_sb, 0)
                    elif hh == 9:
                        norm_into(hq, xq_sb, NS)

                # exchange K halves while V projection runs
                nc.gpsimd.collective_compute(
                    "AllGather", mybir.AluOpType.bypass,
                    replica_groups=PAIRS,
                    ins=[k_half[:, :, :].opt()],
                    outs=[k_gath[:, :, :, :].opt()])

            # ---- V projection for this half ----
            with tc.tile_pool(name="pbv2", bufs=3) as pb2, \
                 tc.tile_pool(name="pbw", bufs=2) as pbw, \
                 tc.tile_pool(name="pb_v", bufs=3, space="PSUM") as pb_v:
                for vs in range(NVS):
                    v_half = v_halfA if vs < NVH else v_halfB
                    wv = pbw.tile([P, DT, NS], F16, tag="wv")
                    nc.sync.dma_start(out=wv[:], in_=wv_in[vs])
                    for kt in range(KTH):
                        tb, off = divmod(kt * P, NS)
                        vps = pb_v.tile([P, NS], F32, tag="vps")
                        for dt in range(DT):
                            nc.tensor.matmul(
                                vps[:], h_blk[tb][:, dt, off:off + P],
                                wv[:, dt, :],
                                start=(dt == 0), stop=(dt == DT - 1))
                        vcp = pb2.tile([P, NS], F16, tag="vcp")
                        nc.scalar.copy(vcp[:], vps[:])
                        nc.sync.dma_start(out=v_half[vs % NVH, kt],
                                          in_=vcp[:])
                    if vs == NVH - 1:
                        # first half of V (heads 0..7) exchanges while the
                        # second half projects
                        nc.gpsimd.collective_compute(
                            "AllGather", mybir.AluOpType.bypass,
                            replica_groups=PAIRS,
                            ins=[v_halfA[:, :, :, :].opt()],
                            outs=[v_gathA[:, :, :, :, :].opt()])
                nc.gpsimd.collective_compute(
                    "AllGather", mybir.AluOpType.bypass,
                    replica_groups=PAIRS,
                    ins=[v_halfB[:, :, :, :].opt()],
                    outs=[v_gathB[:, :, :, :, :].opt()])
        ph.release()

        # ================= P2: Q projection + causal attention ===========
        po = tc.alloc_tile_pool(name="o", bufs=1)        # o (P2 -> P3)
        o_w = [po.tile([P, H, NS], F16, tag=f"o{ws}", name=f"o{ws}")
               for ws in range(NQS)]
        with nc.named_scope("P2"):
            with tc.tile_pool(name="p2", bufs=3) as p2, \
                 tc.tile_pool(name="p2kv", bufs=2) as p2kv, \
                 tc.tile_pool(name="p2pex", bufs=6) as p2pex, \
                 tc.tile_pool(name="p2w", bufs=2) as p2w, \
                 tc.tile_pool(name="p2sc", bufs=4, space="PSUM") as p2sc, \
                 tc.tile_pool(name="p2qp", bufs=1, space="PSUM") as p2qp, \
                 tc.tile_pool(name="p2acc", bufs=2, space="PSUM") as p2acc:

                def qproj_fillers(hh):
                    """Q projection for head hh as a list of single-matmul
                    thunks; the attention loop pumps them into the PE stream
                    between score/AV matmuls so the PE never starves while
                    the exp chain catches up."""
                    wq = p2w.tile([P, DT, P], F16, tag="wq")
                    nc.sync.dma_start(out=wq[:], in_=wq_in[hh])
                    qps = p2qp.tile([P, NQS * NS], F32, tag="qps")
                    thunks = []
                    for ws in range(NQS):
                        for dt in range(DT):
                            def mm(ws=ws, dt=dt):
                                nc.tensor.matmul(
                                    qps[:, ws * NS:(ws + 1) * NS],
                                    wq[:, dt, :],
                                    hq[:, dt, ws * NS:(ws + 1) * NS],
                                    start=(dt == 0), stop=(dt == DT - 1))
                            thunks.append(mm)
                    return qps, thunks

                def qh_copy(qps):
                    qh = p2.tile([P, QT], F16, tag="qh")
                    nc.scalar.copy(qh[:], qps[:])
                    return qh

                # head 0's Q projection runs up front (covers gather wait)
                qps0, th0 = qproj_fillers(0)
                for th in th0:
                    th()
                qh_cur = qh_copy(qps0)

                for hh in range(H):
                    kh = p2kv.tile([P, T], F16, tag="kh")
                    for g in range(2):
                        nc.sync.dma_start(
                            out=kh[:, g * KTH * P:(g + 1) * KTH * P],
                            in_=k_gath[g, hh])
                    vh = p2kv.tile([P, KT, P], F16, tag="vh")
                    voff = (hh % HPS) * P
                    v_gath = v_gathA if hh < H // 2 else v_gathB
                    vslab = (hh // HPS) % NVH
                    for g in range(2):
                        nc.sync.dma_start(
                            out=vh[:, g * KTH:(g + 1) * KTH, :],
                            in_=v_gath[g, vslab].rearrange(
                                "t p c -> p t c")[:, :, voff:voff + P])
                    qh = qh_cur
                    if hh + 1 < H:
                        qps_n, fillers = qproj_fillers(hh + 1)
                    else:
                        qps_n, fillers = None, []

                    def pump(n):
                        for _ in range(min(n, len(fillers))):
                            fillers.pop(0)()

                    for qs in range(NQS):
                        nkt = cfg.nkt(qs)
                        kt0 = cfg.kt0(qs)
                        oacc = p2acc.tile([P, NS], F32, tag="oacc")
                        dsum = p2.tile([P, NS], F16, tag="dsum")
                        scs = {}

                        def emit_sc(kt):
                            sc = p2sc.tile([P, NS], F32, tag="sc")
                            nc.tensor.matmul(
                                sc[:], kh[:, kt * P:(kt + 1) * P],
                                qh[:, qs * NS:(qs + 1) * NS],
                                start=True, stop=True)
                            scs[kt] = sc

                        def emit_rest(kt):
                            sc = scs.pop(kt)
                            pex = p2pex.tile([P, NS], F16, tag="pex")
                            nc.scalar.activation(pex[:], sc[:], AF.Exp,
                                                 scale=cfg.ISQ)
                            if kt >= kt0:
                                pexm = p2pex.tile([P, NS], F16, tag="pexm")
                                nc.vector.tensor_tensor(
                                    pexm[:], pex[:], m01[:, kt - kt0, :],
                                    ALU.mult)
                                pex = pexm
                            if kt == 0:
                                nc.vector.tensor_scalar_add(dsum[:], pex[:],
                                                            0.0)
                            else:
                                nc.vector.tensor_tensor(
                                    dsum[:], dsum[:], pex[:], ALU.add)
                            nc.tensor.matmul(
                                oacc[:], vh[:, kt, :], pex[:],
                                start=(kt == 0), stop=(kt == nkt - 1))

                        emit_sc(0)
                        emit_sc(1)
                        emit_sc(2)
                        for kt in range(nkt):
                            if kt + 3 < nkt:
                                emit_sc(kt + 3)
                            emit_rest(kt)
                            pump(2)
                        # softmax denominator: ones[128x128] matmul sums the
                        # partition dim AND broadcasts in one PE op.  Lives
                        # in the oacc ring so the score ring keeps 3-ahead
                        # staging depth.
                        dbc_t = p2acc.tile([P, NS], F32, tag="oacc")
                        nc.tensor.matmul(dbc_t[:], ones_sq[:], dsum[:],
                                         start=True, stop=True)
                        recb = p2.tile([P, NS], F32, tag="recb")
                        nc.vector.reciprocal_approx_fast(recb[:], dbc_t[:])
                        nc.vector.tensor_tensor(o_w[qs][:, hh, :], oacc[:],
                                                recb[:], ALU.mult)
                    for th in fillers:
                        th()
                    if qps_n is not None:
                        qh_cur = qh_copy(qps_n)

        # ================= P3: out-proj + residual + norm2 ===============
        # right-side stack: lifetime (P3 -> P5) crosses po's release
        px2 = tc.alloc_tile_pool(name="x2h2", bufs=1, side="right")
        x2 = px2.tile([P, NDCT, QT], F16, tag="x2")
        h2 = px2.tile([P, DT, QT], F16, tag="h2")
        with nc.named_scope("P3"):
            with tc.tile_pool(name="p3", bufs=2) as p3, \
                 tc.tile_pool(name="p3w", bufs=2) as p3w, \
                 tc.tile_pool(name="p3mm", bufs=2, space="PSUM") as p3mm, \
                 tc.tile_pool(name="p3s", bufs=1, space="PSUM") as p3s, \
                 tc.tile_pool(name="p3bc", bufs=2, space="PSUM") as p3bc:
                ssp2 = p3s.tile([1, QT], F32, tag="ssp2")

                def stat2(dct, sq2):
                    for ws in range(NQS):
                        nc.tensor.matmul(ssp2[:, ws * NS:(ws + 1) * NS],
                                         ones_c[:],
                                         sq2[:, ws * NS:(ws + 1) * NS],
                                         start=(dct == 0),
                                         stop=(dct == NDCT - 1))

                pend = []
                for dct in range(NDCT):
                    wo = p3w.tile([P, H, P], F16, tag="wo")
                    nc.sync.dma_start(out=wo[:], in_=wo_in[dct])
                    ops = p3mm.tile([P, NQS, NS], F32, tag="ops")
                    for ws in range(NQS):
                        for hh in range(H):
                            nc.tensor.matmul(
                                ops[:, ws, :], wo[:, hh, :], o_w[ws][:, hh, :],
                                start=(hh == 0), stop=(hh == H - 1))
                    nc.vector.tensor_tensor(
                        x2[:, dct, :],
                        ops.rearrange("p a b -> p (a b)"),
                        xq_sb[:, dct, :], ALU.add)
                    sq2 = p3.tile([P, QT], F16, tag="sq2", bufs=4)
                    nc.scalar.activation(sq2[:], x2[:, dct, :], AF.Square)
                    # delay the tiny stat matmuls two dcts so the PE isn't
                    # held hostage to the DVE->Act chain of the current dct
                    pend.append((dct, sq2))
                    if len(pend) >= 3:
                        stat2(*pend.pop(0))
                for it in pend:
                    stat2(*it)
                for ws in range(NQS):
                    q0 = ws * NS
                    srow2 = p3.tile([1, NS], F32, tag="srow2")
                    nc.scalar.activation(srow2[:], ssp2[:, q0:q0 + NS],
                                         AF.Sqrt, scale=1.0 / D, bias=epsT[:])
                    rec232 = p3.tile([1, NS], F32, tag="rec232")
                    nc.vector.reciprocal_approx_fast(rec232[:], srow2[:])
                    rec2 = p3.tile([1, NS], F16, tag="rec2")
                    nc.scalar.copy(rec2[:], rec232[:])
                    bc2 = p3bc.tile([P, NS], F32, tag="bc2")
                    nc.tensor.matmul(bc2[:], ones_r[:], rec2[:],
                                     start=True, stop=True)
                    bc2s = p3.tile([P, NS], F16, tag="bc2s")
                    nc.scalar.copy(bc2s[:], bc2[:])
                    for dt in range(DT):
                        nc.vector.tensor_tensor(h2[:, dt, q0:q0 + NS],
                                                x2[:, dt, q0:q0 + NS],
                                                bc2s[:], ALU.mult)
        po.release()
        pax.release()

        # ================= P5: SwiGLU MLP + residual =====================
        with nc.named_scope("P5"):
            with tc.tile_pool(name="p5", bufs=2) as p5, \
                 tc.tile_pool(name="p5w", bufs=2) as p5w, \
                 tc.tile_pool(name="p5mt", bufs=1) as p5mt, \
                 tc.tile_pool(name="p5gu", bufs=2, space="PSUM") as p5gu, \
                 tc.tile_pool(name="p5d", bufs=3, space="PSUM") as p5d:
                for ws in range(NQS):
                    q0 = ws * NS
                    mt = p5mt.tile([P, NFT, NS], F16, tag="mt")
                    for ft in range(NFT):
                        wg = p5w.tile([P, DT, P], F16, tag="wg")
                        nc.sync.dma_start(out=wg[:], in_=wg_in[ft])
                        wu = p5w.tile([P, DT, P], F16, tag="wu")
                        nc.sync.dma_start(out=wu[:], in_=wu_in[ft])
                        guw = p5gu.tile([P, 2, NS], F32, tag="guw")
                        for dt in range(DT):
                            nc.tensor.matmul(
                                guw[:, 0, :], wg[:, dt, :],
                                h2[:, dt, q0:q0 + NS],
                                start=(dt == 0), stop=(dt == DT - 1))
                        for dt in range(DT):
                            nc.tensor.matmul(
                                guw[:, 1, :], wu[:, dt, :],
                                h2[:, dt, q0:q0 + NS],
                                start=(dt == 0), stop=(dt == DT - 1))
                        sg = p5.tile([P, NS], F16, tag="sg")
                        nc.scalar.activation(sg[:], guw[:, 0, :], AF.Silu)
                        nc.vector.tensor_tensor(mt[:, ft, :], sg[:],
                                                guw[:, 1, :], ALU.mult)
                    for dct in range(NDCT):
                        wd = p5w.tile([P, NFT, P], F16, tag="wd")
                        nc.sync.dma_start(out=wd[:], in_=wd_in[dct])
                        dps = p5d.tile([P, NS], F32, tag="dacc")
                        for ft in range(NFT):
                            nc.tensor.matmul(dps[:], wd[:, ft, :],
                                             mt[:, ft, :],
                                             start=(ft == 0),
                                             stop=(ft == NFT - 1))
                        yt = p5.tile([P, NS], F32, tag="yt")
                        nc.vector.tensor_tensor(yt[:], dps[:],
                                                x2[:, dct, q0:q0 + NS],
                                                ALU.add)
                        nc.sync.dma_start(out=y_out[dct][:, q0:q0 + NS],
                                          in_=yt[:])
        px2.release()
        pc.release()

    nc.compile()
    return nc


# --------------------------------------------------------------------------
# Host side
# --------------------------------------------------------------------------

_NC_CACHE = {}


def _get_nc(cfg):
    key = (cfg.D, cfg.T, cfg.FF, cfg.QT)
    if key not in _NC_CACHE:
        _NC_CACHE[key] = build(cfg)
    return _NC_CACHE[key]


def _tile_lhs(a, ncols):
    # [Din, Cout] -> [Cout/ncols, P, Din/P, ncols]
    d, c = a.shape
    return np.ascontiguousarray(
        a.reshape(d // P, P, c // ncols, ncols).transpose(2, 1, 0, 3))


def prep_weights(cfg, w_qkv, w_out, w_gate, w_up, w_down, ln1, ln2):
    D, TD, FF, NS = cfg.D, cfg.TD, cfg.FF, cfg.NS
    f32, f16 = np.float32, np.float16
    w_qkv_f = (np.asarray(w_qkv, f32) * np.asarray(ln1, f32)[None, :])
    wqT = w_qkv_f[0:TD].T
    wkT = w_qkv_f[TD:2 * TD].T
    wvT = w_qkv_f[2 * TD:3 * TD].T
    woT = np.asarray(w_out, f32).T            # [TD, D]
    wgT = (np.asarray(w_gate, f32) * np.asarray(ln2, f32)[None, :]).T
    wuT = (np.asarray(w_up, f32) * np.asarray(ln2, f32)[None, :]).T
    wdT = np.asarray(w_down, f32).T           # [FF, D]

    wd_in = np.ascontiguousarray(
        wdT.reshape(cfg.NFT, P, cfg.NDCT, P).transpose(2, 1, 0, 3))
    return dict(
        wq_in=_tile_lhs(wqT, P).astype(f16),
        wk_in=_tile_lhs(wkT, P).astype(f16),
        wv_in=_tile_lhs(wvT, NS).astype(f16),
        wo_in=_tile_lhs(woT, P).astype(f16),
        wg_in=_tile_lhs(wgT, P).astype(f16),
        wu_in=_tile_lhs(wuT, P).astype(f16),
        wd_in=wd_in.astype(f16),
    )


def prep_core_inputs(cfg, xb, parity, wdict):
    """Per-core tensors for batch slice xb [T, D]; query tokens are the
    interleaved slice parity::stride; K/V tokens are the contiguous half
    [parity*T/2, (parity+1)*T/2)."""
    T, D, QT, NS = cfg.T, cfg.D, cfg.QT, cfg.NS
    stride = cfg.stride
    f16 = np.float16
    half = T // 2
    xT = np.ascontiguousarray(np.asarray(xb, np.float32).T)   # [D, T]
    x_in = np.ascontiguousarray(
        xT[:, parity * half:(parity + 1) * half]).reshape(
        cfg.DT, P, half).astype(f16)
    xq_in = np.ascontiguousarray(
        xT[:, parity::stride]).reshape(cfg.DT, P, QT).astype(f16)
    # 0/1 mask for the NSLOT diagonal k tiles of every query slice:
    # slot s, row ki, col q allowed iff 128*s + ki <= stride*q + parity
    ki = np.arange(P)[:, None, None]
    s = np.arange(cfg.NSLOT)[None, :, None]
    q = np.arange(NS)[None, None, :]
    m01_in = ((P * s + ki) <= (stride * q + parity)).astype(f16)
    out = dict(x_in=x_in, xq_in=xq_in, m01_in=m01_in)
    out.update(wdict)
    return out


def run(cfg, x, w_qkv, w_out, w_gate, w_up, w_down, ln1, ln2):
    nc = _get_nc(cfg)
    wdict = prep_weights(cfg, w_qkv, w_out, w_gate, w_up, w_down, ln1, ln2)
    x = np.asarray(x, np.float32)
    Bc = x.shape[0]
    in_maps = []
    for c in range(N_CORES):
        b, half = divmod(c, 2)
        in_maps.append(prep_core_inputs(cfg, x[b % Bc], half, wdict))
    res = run_bass_kernel_spmd(nc, in_maps, list(range(N_CORES)))
    y = np.empty((Bc, cfg.T, cfg.D), np.float32)
    for c in range(N_CORES):
        b, parity = divmod(c, 2)
        if b < Bc:
            yc = res.results[c]["y_out"].reshape(cfg.D, cfg.QT)
            y[b, parity::cfg.stride, :] = yc.T
    return y


def kernel(x, w_qkv, w_out, w_gate, w_up, w_down, ln1, ln2):
    return run(FULL, x, w_qkv, w_out, w_gate, w_up, w_down, ln1, ln2)
